# revision 25
# baseline (speedup 1.0000x reference)
"""Trainium2 Bass kernel for the BayesRing RNN problem.

Math: the reference's (N,N) weight matrices W_even, W_odd, M are all built
from cos/sin of pairwise phase differences, so they are rank-2 in the
{cos(phi), sin(phi)} basis.  Since r0 = kappa0*cos(phi) starts in that basis,
the whole state r_t = A_t*cos(phi) + B_t*sin(phi) stays rank-2 forever, and
the per-step update collapses (using sum cos^2 = sum sin^2 = N/2,
sum sin*cos = 0) to a per-batch 2-scalar recurrence:

    amp_t   = sqrt(A_t^2 + B_t^2)
    g_t     = C1 - C2*amp_t                  (C1 = 1 - ALPHA*DT + DT*a_even)
    A_{t+1} = g_t*A_t - a_odd*av_t*B_t + KZ*cos(hd_t)
    B_{t+1} = g_t*B_t + a_odd*av_t*A_t + KZ*sin(hd_t)

The output is out[b,t,i] = A_{t+1}[b]*cos(phi_i) + B_{t+1}[b]*sin(phi_i).

On device (per core: 128 batch rows on the 128 partitions):
  1. bulk-compute u = KZ*cos(hd), v = KZ*sin(hd), gi = a_odd*av
  2. solve the nonlinear recurrence with damped-Picard sweeps, each sweep
     being bulk DVE/ACT ops on (128,1500) tiles plus two hardware
     tensor_tensor_scan instructions (state = g*state + c)
  3. expand to (128,1500,80) with PE matmuls: transpose A/B time-blocks onto
     the contraction axis and multiply by a block-diagonal cos/sin basis,
     then stream PSUM -> SBUF -> DRAM.

Sharding: data-parallel over batch, B=1024 -> 8 cores x 128.
"""

import math

import numpy as np

import concourse.bacc as bacc
import concourse.tile as tile
from concourse import mybir
from concourse.bass_utils import run_bass_kernel_spmd

# ---------------------------------------------------------------------------
# Problem constants (hardcoded; must match the reference)
# ---------------------------------------------------------------------------
N = 80
B = 1024
T = 1500
DT = 0.01
KP = 1.0
KV = 2.0
KAPPA_0 = 10.0
ALPHA = 0.5 * (KV / KP) * (1.0 / (KP + KV))
A_EVEN = 1.0 / (KP + KV)
A_ODD = KV / (KP + KV)
C2 = DT / (KP + KV)
C1 = 1.0 - ALPHA * DT + DT * A_EVEN

N_CORES = 8
P = B // N_CORES  # 128 batch rows per core

# Picard solver parameters (validated offline: rel err ~8e-5 at 8 sweeps)
SWEEPS = 8
THETA = 0.7
AMP_INIT = 3.6

# Expansion tiling: supergroups of 60 timesteps, 10 matmuls of 6 steps each
SG = 60
N_SG = T // SG  # 25
MM_T = 6
MM_PER_SG = SG // MM_T  # 10
MM_N = MM_T * N  # 480 output columns per matmul


def _kappa_z():
    """xi_fun_inv(15*DT) by bisection, same algorithm as the reference."""

    def f(a):
        x = (a / 2.0) ** 2
        t0, t1 = 1.0, a / 2.0
        i0, i1 = t0, t1
        for k in range(1, 30):
            t0 *= x / (k * k)
            t1 *= x / (k * (k + 1))
            i0 += t0
            i1 += t1
        return a * i1 / i0 - 15 * DT

    lo, hi = 1e-3, 50.0
    for _ in range(200):
        mid = 0.5 * (lo + hi)
        if f(lo) * f(mid) <= 0.0:
            hi = mid
        else:
            lo = mid
    return 0.5 * (lo + hi)


KAPPA_Z = _kappa_z()


def _host_constants():
    """Basis matrix for the expansion matmuls + identity for PE transpose."""
    phi = np.linspace(-np.pi, np.pi - 2 * np.pi / N, N).astype(np.float32)
    cosb = np.cos(phi.astype(np.float64)).astype(np.float32)
    sinb = np.sin(phi.astype(np.float64)).astype(np.float32)
    # basis[k, 480*j + 80*tau + i]:
    #   row 6j+tau      -> cos_i   (multiplies A at local step 6j+tau)
    #   row 64+6j+tau   -> sin_i   (multiplies B at local step 6j+tau)
    # rows 60..63 and 124..127 are zero (engine partition offsets must be
    # multiples of 32, so the B block sits at partition 64).
    basis = np.zeros((128, MM_PER_SG * MM_N), dtype=np.float32)
    for j in range(MM_PER_SG):
        for tau in range(MM_T):
            col0 = j * MM_N + tau * N
            basis[j * MM_T + tau, col0 : col0 + N] = cosb
            basis[64 + j * MM_T + tau, col0 : col0 + N] = sinb
    ident = np.eye(128, dtype=np.float32)
    return basis, ident


def build_bass():
    f32 = mybir.dt.float32
    f32r = mybir.dt.float32r
    AF = mybir.ActivationFunctionType
    OP = mybir.AluOpType

    nc = bacc.Bacc(None, target_bir_lowering=False)
    x_ext = nc.declare_dram_parameter("x", [P, T, 2], f32, isOutput=False)
    basis_ext = nc.declare_dram_parameter(
        "basis", [128, MM_PER_SG * MM_N], f32r, isOutput=False
    )
    ident_ext = nc.declare_dram_parameter("ident", [128, 128], f32, isOutput=False)
    out_ext = nc.declare_dram_parameter("out", [P, T, N], f32, isOutput=True)

    TWO_PI = 2.0 * math.pi
    PI = math.pi

    with tile.TileContext(nc) as tc:
        with (
            tc.tile_pool(name="singles", bufs=1) as singles,
            tc.tile_pool(name="zt", bufs=2) as zt_pool,
            tc.tile_pool(name="stage", bufs=8) as stage_pool,
            tc.tile_pool(name="ptrans", bufs=2, space="PSUM") as ptrans_pool,
            tc.tile_pool(name="pmm", bufs=4, space="PSUM") as pmm_pool,
        ):
            # ---- load inputs / constants -------------------------------
            xt = singles.tile([P, 2 * T], f32)
            nc.sync.dma_start(out=xt[:], in_=x_ext[:].rearrange("b t c -> b (t c)"))
            basis_sb = singles.tile([128, MM_PER_SG * MM_N], f32r)
            nc.sync.dma_start(out=basis_sb[:], in_=basis_ext[:])
            ident_sb = singles.tile([128, 128], f32)
            nc.sync.dma_start(out=ident_sb[:], in_=ident_ext[:])

            xv = xt[:].rearrange("b (t c) -> b t c", c=2)
            hd = xv[:, :, 0]
            av = xv[:, :, 1]



            # ---- setup: u = KZ*cos(hd), v = KZ*sin(hd), gi = a_odd*av --
            u = singles.tile([P, T], f32)
            v = singles.tile([P, T], f32)
            gi = singles.tile([P, T], f32)
            w1 = singles.tile([P, T], f32)  # reused as sqA later
            w2 = singles.tile([P, T], f32)  # reused as sqB later

            # range-reduce into [-pi, pi]: x - 2pi*round(x/2pi), with
            # round() done by the f32 magic-number trick (+1.5*2^23 then -).
            MAGIC = 12582912.0  # 1.5 * 2**23
            INV2PI = 1.0 / TWO_PI
            # cos(hd) = sin(hd + pi/2): shift first, then reduce
            nc.vector.tensor_scalar(
                out=u[:], in0=hd, scalar1=0.5 * PI, scalar2=None, op0=OP.add
            )
            nc.vector.tensor_scalar(
                out=w1[:], in0=u[:], scalar1=INV2PI, scalar2=MAGIC,
                op0=OP.mult, op1=OP.add,
            )
            nc.vector.tensor_scalar(
                out=w1[:], in0=w1[:], scalar1=MAGIC, scalar2=None, op0=OP.subtract
            )
            nc.vector.scalar_tensor_tensor(
                out=w1[:], in0=w1[:], scalar=-TWO_PI, in1=u[:],
                op0=OP.mult, op1=OP.add,
            )
            nc.scalar.activation(out=u[:], in_=w1[:], func=AF.Sin)
            # sin(hd)
            nc.vector.tensor_scalar(
                out=w2[:], in0=hd, scalar1=INV2PI, scalar2=MAGIC,
                op0=OP.mult, op1=OP.add,
            )
            nc.vector.tensor_scalar(
                out=w2[:], in0=w2[:], scalar1=MAGIC, scalar2=None, op0=OP.subtract
            )
            nc.vector.scalar_tensor_tensor(
                out=w2[:], in0=w2[:], scalar=-TWO_PI, in1=hd,
                op0=OP.mult, op1=OP.add,
            )
            nc.scalar.activation(out=v[:], in_=w2[:], func=AF.Sin)
            nc.vector.tensor_scalar(
                out=u[:], in0=u[:], scalar1=KAPPA_Z, scalar2=None, op0=OP.mult
            )
            nc.vector.tensor_scalar(
                out=v[:], in0=v[:], scalar1=KAPPA_Z, scalar2=None, op0=OP.mult
            )
            nc.vector.tensor_scalar(
                out=gi[:], in0=av, scalar1=A_ODD, scalar2=None, op0=OP.mult
            )

            # ---- Picard iteration --------------------------------------
            # A and B trajectories, each padded by 4 columns so the expansion
            # can transpose 64-wide blocks (engine partition offsets must be
            # multiples of 32; the 4 extra rows are zeroed in the basis).
            W = T + 1 + 4
            ABbuf = singles.tile([P, 2 * W], f32)
            Abuf = ABbuf[:, 0:W]
            Bbuf = ABbuf[:, W : 2 * W]
            gd = singles.tile([P, T], f32)
            amp2 = singles.tile([P, T], f32)
            gnew = singles.tile([P, T], f32)
            cA = singles.tile([P, T], f32)
            cB = singles.tile([P, T], f32)

            nc.vector.memset(Abuf[:, 0:1], KAPPA_0)
            nc.vector.memset(Bbuf[:, 0:1], 0.0)
            nc.vector.memset(Abuf[:, T + 1 : W], 0.0)
            nc.vector.memset(Bbuf[:, T + 1 : W], 0.0)
            nc.vector.memset(gd[:], C1 - C2 * AMP_INIT)

            # sweep 0: trajectory guess A=AMP_INIT, B=0
            nc.vector.scalar_tensor_tensor(
                out=cB[:], in0=gi[:], scalar=AMP_INIT, in1=v[:],
                op0=OP.mult, op1=OP.add,
            )
            nc.vector.tensor_tensor_scan(
                out=Abuf[:, 1 : T + 1], data0=gd[:], data1=u[:],
                initial=KAPPA_0, op0=OP.mult, op1=OP.add,
            )
            nc.vector.tensor_tensor_scan(
                out=Bbuf[:, 1 : T + 1], data0=gd[:], data1=cB[:],
                initial=0.0, op0=OP.mult, op1=OP.add,
            )

            for _ in range(1, SWEEPS):
                Ae = Abuf[:, 0:T]
                Be = Bbuf[:, 0:T]
                sqA, sqB = w1, w2
                nc.scalar.activation(out=sqA[:], in_=Ae, func=AF.Square)
                nc.scalar.activation(out=sqB[:], in_=Be, func=AF.Square)
                nc.vector.tensor_tensor(out=amp2[:], in0=sqA[:], in1=sqB[:], op=OP.add)
                # gnew = THETA*(C1 - C2*sqrt(amp2)) ; then gd = (1-THETA)*gd + gnew
                nc.scalar.activation(out=gnew[:], in_=amp2[:], func=AF.Sqrt)
                nc.scalar.activation(
                    out=gnew[:], in_=gnew[:], func=AF.Copy,
                    scale=-THETA * C2, bias=THETA * C1,
                )
                nc.vector.scalar_tensor_tensor(
                    out=gd[:], in0=gd[:], scalar=1.0 - THETA, in1=gnew[:],
                    op0=OP.mult, op1=OP.add,
                )
                # cA = u - gi*B ; cB = v + gi*A
                nc.vector.tensor_tensor(out=cA[:], in0=gi[:], in1=Be, op=OP.mult)
                nc.vector.tensor_tensor(out=cA[:], in0=u[:], in1=cA[:], op=OP.subtract)
                nc.vector.tensor_tensor(out=cB[:], in0=gi[:], in1=Ae, op=OP.mult)
                nc.vector.tensor_tensor(out=cB[:], in0=v[:], in1=cB[:], op=OP.add)
                nc.vector.tensor_tensor_scan(
                    out=Abuf[:, 1 : T + 1], data0=gd[:], data1=cA[:],
                    initial=KAPPA_0, op0=OP.mult, op1=OP.add,
                )
                nc.vector.tensor_tensor_scan(
                    out=Bbuf[:, 1 : T + 1], data0=gd[:], data1=cB[:],
                    initial=0.0, op0=OP.mult, op1=OP.add,
                )

            # ---- expansion: out[b,t,i] = A[t+1]*cos_i + B[t+1]*sin_i ----
            out_flat = out_ext[:].rearrange("b t i -> b (t i)")
            for g in range(N_SG):
                t0 = 1 + g * SG  # post-step states live at Abuf[:, 1:]
                tpA = ptrans_pool.tile([64, 128], f32, tag="tp")
                tpB = ptrans_pool.tile([64, 128], f32, tag="tp")
                nc.tensor.transpose(
                    out=tpA[:], in_=Abuf[:, t0 : t0 + 64], identity=ident_sb[:]
                )
                nc.tensor.transpose(
                    out=tpB[:], in_=Bbuf[:, t0 : t0 + 64], identity=ident_sb[:]
                )
                zt = zt_pool.tile([128, 128], f32r)
                nc.vector.tensor_copy(out=zt[0:64, :], in_=tpA[:])
                nc.vector.tensor_copy(out=zt[64:128, :], in_=tpB[:])
                for j in range(MM_PER_SG):
                    pm = pmm_pool.tile([P, MM_N], f32)
                    nc.tensor.matmul(
                        out=pm[:],
                        lhsT=zt[:],
                        rhs=basis_sb[:, j * MM_N : (j + 1) * MM_N],
                        start=True,
                        stop=True,
                    )
                    st = stage_pool.tile([P, MM_N], f32)
                    if j % 2 == 0:
                        nc.scalar.activation(out=st[:], in_=pm[:], func=AF.Copy)
                    else:
                        nc.vector.tensor_copy(out=st[:], in_=pm[:])
                    col0 = (g * MM_PER_SG + j) * MM_N
                    nc.gpsimd.dma_start(
                        out=out_flat[:, col0 : col0 + MM_N], in_=st[:]
                    )

    nc.finalize()
    return nc


_NC_CACHE = None


def _get_nc():
    global _NC_CACHE
    if _NC_CACHE is None:
        _NC_CACHE = build_bass()
    return _NC_CACHE


def kernel(inputs: np.ndarray, **extra):
    """inputs: (1024, 1500, 2) float32 -> (out (1024,1500,80), r_final (1,1024,80))."""
    assert inputs.shape == (B, T, 2), inputs.shape
    x = np.ascontiguousarray(np.asarray(inputs, dtype=np.float32))
    basis, ident = _host_constants()
    nc = _get_nc()
    in_maps = [
        {
            "x": np.ascontiguousarray(x[c * P : (c + 1) * P]),
            "basis": basis,
            "ident": ident,
        }
        for c in range(N_CORES)
    ]
    res = run_bass_kernel_spmd(nc, in_maps, list(range(N_CORES)))
    out = np.concatenate([res.results[c]["out"] for c in range(N_CORES)], axis=0)
    r_final = out[:, -1, :][None]
    return out, r_final
